# revision 34
# baseline (speedup 1.0000x reference)
"""Trainium2 Bass kernel for CorpusSupportSets RBF tangent-field.

Math per sample row i (dim 768), one-hot mask selects dipole k:
    t_j  = z . s_j                      (unit z, unit s_j)
    m_j  = a_j g_j e^{-g_j(2-2t_j)} = C_j exp(2 g_j t_j),  C_j = a_j g_j e^{-2 g_j}
    beta = -(m0 t0 + m1 t1)
    p    = beta z + m0 s0 + m1 s1
    |p|^2 = m0^2 + m1^2 - beta^2 + 2 m0 m1 (s0.s1)
    out  = p / |p|

Sharding: data-parallel over batch across 8 cores (2048 rows each).

Host prep (layout/dtype only + per-table-row constants): z in bf16
partition-major [128, 16, 768]; per-sample table row index as u32
[128, 16]; table rows [s0|s1|C0 C1 2g0 2g1 c01|pad] bf16; output bf16
partition-major, upcast to f32 on host.

Device structure (16 tiles of 128 rows, 4 groups of 4 tiles):
- per group: one sync HWDGE z load, one batched indirect DMA gathering
  4x128 table rows (gpsimd SWDGE).
- dots t_j: scalar_tensor_tensor with accum_out, split DVE/Pool.
- per-group small math on [128,4,2]: exp on ACT (the only ACT use),
  1/|p| via bit-magic rsqrt + 2 Newton steps on DVE.
- p = diag(bp) z + diag(m0p) s0 + diag(m1p) s1 as accumulating PE
  matmuls with diag stationaries built on DVE (eye * per-partition
  scalar); PSUM->SBUF bf16 copies alternate DVE/Pool.
"""
import sys

for _p in ("/opt/trn_rl_repo",):
    if _p not in sys.path:
        sys.path.insert(0, _p)

import numpy as np

import concourse.bass as bass
import concourse.tile as tile
from concourse import mybir
from concourse.bass_utils import run_bass_kernel_spmd
from concourse.vector_clock import ScopedClock

# ---------------------------------------------------------------------------
# Workaround: this walrus build only accepts ONE semaphore wait per
# instruction; the TileContext exit drain accumulates one wait per live
# semaphore lane.  Split overflow waits onto trailing sync-engine NOPs.
_MAX_WAITS = 1


def _split_waits(nc, inst):
    si = inst.sync_info
    if si is None:
        return
    waits = list(si.on_wait)
    if len(waits) <= _MAX_WAITS:
        return
    inst.sync_info = mybir.SyncInfo(
        on_wait=waits[:_MAX_WAITS], on_update=list(si.on_update)
    )
    for i in range(_MAX_WAITS, len(waits), _MAX_WAITS):
        nop = nc.sync.nop(nofuse=True, hint="drain_wait_overflow")
        nop.ins.sync_info = mybir.SyncInfo(
            on_wait=waits[i : i + _MAX_WAITS], on_update=[]
        )


def _patched_drain_and_barrier(self, tick_clock, wait_clock):
    drain_inst = self.nc.sync.drain()
    wait_clock.add_sem_waits(
        drain_inst.ins, ScopedClock({None: tick_clock.global_clock})
    )
    _split_waits(self.nc, drain_inst.ins)
    self.nc.all_engine_barrier()
    assert self.sems is not None
    popped = self.nc._tile_sem_poison_stack.pop()
    assert popped is self._sem_poison
    self.nc.clear_and_free_semaphores(list(self.sems.allocated().values()))
    self.nc.all_engine_barrier()


_orig_commit = tile.TileContext._commit_instruction


def _patched_commit(self, inst, lazy_reg_writes=True):
    si = getattr(inst, "sync_info", None)
    if (
        si is not None
        and si.on_wait
        and len(si.on_wait) > _MAX_WAITS
        and inst.engine != mybir.EngineType.Unassigned
    ):
        waits = list(si.on_wait)
        inst.sync_info = mybir.SyncInfo(
            on_wait=waits[:_MAX_WAITS], on_update=list(si.on_update)
        )
        for _i, _w in enumerate(waits[_MAX_WAITS:]):
            nop = mybir.InstNoOp(
                name=f"{inst.name}_w{_i}",
                engine=inst.engine,
                sync_info=mybir.SyncInfo(on_wait=[_w], on_update=[]),
                bass_nofuse=True,
            )
            self._add_instruction(nop)
    return _orig_commit(self, inst, lazy_reg_writes)


tile.TileContext._drain_and_barrier = _patched_drain_and_barrier
tile.TileContext._commit_instruction = _patched_commit

# ---------------------------------------------------------------------------
BS, K, DIM = 16384, 1000, 768
NCORES = 8
ROWS = BS // NCORES  # 2048 rows per core
P = 128
NT = ROWS // P  # 16 tiles of 128 rows
GRP = 4  # tiles per group
NG = NT // GRP  # 4 groups
TBL_W = 2 * DIM + 8  # 1544 bf16 cols = 3088B per row
PC = 2 * DIM  # C0, C1, 2g0, 2g1, c01 start here
F32 = mybir.dt.float32
BF16 = mybir.dt.bfloat16
U32 = mybir.dt.uint32
MAGIC = 0x5F3759DF


def build_nc(rows=ROWS):
    NT = rows // P
    HGRP = 8  # tiles per half
    OP = mybir.AluOpType
    AT = mybir.ActivationFunctionType
    nc = bass.Bass()
    zin = nc.dram_tensor("zin", [P, NT, DIM], BF16, kind="ExternalInput")
    idx = nc.dram_tensor("idx", [P, NT], U32, kind="ExternalInput")
    tbl = nc.dram_tensor("tbl", [K, TBL_W], BF16, kind="ExternalInput")
    eye = nc.dram_tensor("eye", [P, P], BF16, kind="ExternalInput")
    magic = nc.dram_tensor("magic", [P, 1], U32, kind="ExternalInput")
    out = nc.dram_tensor("out", [P, NT, DIM], BF16, kind="ExternalOutput")

    with tile.TileContext(nc) as tc:
        with (
            tc.tile_pool(name="zp", bufs=3) as zp,
            tc.tile_pool(name="selp", bufs=3) as selp,
            tc.tile_pool(name="outp", bufs=3) as outp,
            tc.tile_pool(name="scrap", bufs=4) as scrp,
            tc.tile_pool(name="diag", bufs=3) as diagp,
            tc.tile_pool(name="psum", bufs=4, space="PSUM") as psump,
            tc.tile_pool(name="tiny", bufs=40) as tinyp,
            tc.tile_pool(name="singles", bufs=1) as singles,
        ):
            t_a = singles.tile([P, NT, 2], F32)
            idxt = singles.tile([P, NT], U32)
            eyeb = singles.tile([P, P], BF16)
            mg = singles.tile([P, 1], U32)

            # batches of tiles: small first batch so its ACT accumulates /
            # exp / first matmuls start early, small last batch so the
            # serial tail (smalls -> diag -> matmul -> copy -> DMA) is short
            BATCHES = [(0, 6), (6, 12), (12, 16)]

            def loads(lo, hi):
                bn = hi - lo
                z_g = zp.tile([P, bn, DIM], BF16, name="z_g", tag="z")
                nc.sync.dma_start(out=z_g[:], in_=zin[:, lo:hi, :])
                sel = selp.tile([P, bn, TBL_W], BF16, name="sel", tag="sel")
                # multi-column offset APs mis-address on real HW (CoreSim
                # accepts them): one indirect DMA per 128-row tile
                for n in range(bn):
                    t = lo + n
                    nc.gpsimd.indirect_dma_start(
                        out=sel[:, n, :],
                        out_offset=None,
                        in_=tbl[:],
                        in_offset=bass.IndirectOffsetOnAxis(
                            ap=idxt[:, t : t + 1], axis=0
                        ),
                    )
                return dict(lo=lo, bn=bn, z_g=z_g, sel=sel)

            def dots(st):
                lo, bn, z_g, sel = st["lo"], st["bn"], st["z_g"], st["sel"]
                for n in range(bn):
                    t = lo + n
                    # ~6/16 tiles offload both dot accumulates to ACT (2x-mode
                    # DVE multiply + ACT Copy-with-accum); the rest are 1x
                    # stt-with-accum on DVE. Balances DVE vs ACT busy time.
                    # block-contiguous engine assignment so no tile region
                    # is written by two engines: tiles 0-9 ACT accums,
                    # tiles 10-15 DVE stt (ACT is the denser critical path)
                    on_act = t < 10
                    for pole in range(2):
                        if on_act:
                            prod = scrp.tile([P, DIM], BF16, name="prod", tag="scr")
                            nc.vector.tensor_tensor(
                                out=prod[:],
                                in0=z_g[:, n, :],
                                in1=sel[:, n, pole * DIM : (pole + 1) * DIM],
                                op=OP.mult,
                            )
                            pscr = scrp.tile([P, DIM], BF16, name="pscr", tag="scr2")
                            nc.scalar.activation(
                                out=pscr[:],
                                in_=prod[:],
                                func=AT.Copy,
                                accum_out=t_a[:, t, pole : pole + 1],
                            )
                        else:
                            pscr = scrp.tile([P, DIM], BF16, name="pscr", tag="scr2")
                            nc.vector.scalar_tensor_tensor(
                                out=pscr[:],
                                in0=z_g[:, n, :],
                                scalar=1.0,
                                in1=sel[:, n, pole * DIM : (pole + 1) * DIM],
                                op0=OP.mult,
                                op1=OP.mult,
                                accum_out=t_a[:, t, pole : pole + 1],
                            )

            def smalls(st):
                lo, bn, sel = st["lo"], st["bn"], st["sel"]
                tg = t_a[:, lo : lo + bn, :]  # [P,bn,2] f32
                selC = sel[:, :, PC : PC + 2]
                selG2 = sel[:, :, PC + 2 : PC + 4]
                selc01 = sel[:, :, PC + 4 : PC + 5].rearrange("p c o -> p (c o)")
                tp = lambda shape, name: tinyp.tile(shape, F32, name=name, tag="tiny")
                # cf holds the combo coefficients [beta, m0, m1] * 1/|p|
                # per tile, interleaved for the batched diag build
                cf = tinyp.tile([P, bn, 3], F32, name="cf", tag="cf")
                u = tp([P, bn, 2], "u")
                nc.vector.tensor_tensor(out=u[:], in0=selG2, in1=tg, op=OP.mult)
                e = tp([P, bn, 2], "e")
                nc.scalar.activation(out=e[:], in_=u[:], func=AT.Exp)
                m = cf[:, :, 1:3]
                nc.vector.tensor_tensor(out=m, in0=selC, in1=e[:], op=OP.mult)
                hh = tp([P, bn, 2], "hh")
                nc.vector.tensor_tensor(out=hh[:], in0=m, in1=tg, op=OP.mult)
                beta = cf[:, :, 0:1].rearrange("p c o -> p (c o)")
                nc.vector.scalar_tensor_tensor(
                    out=beta, in0=hh[:, :, 0], scalar=-1.0, in1=hh[:, :, 1],
                    op0=OP.mult, op1=OP.subtract,
                )
                m2 = tp([P, bn, 2], "m2")
                nc.vector.tensor_tensor(out=m2[:], in0=m, in1=m, op=OP.mult)
                s2 = tp([P, bn], "s2")
                nc.vector.tensor_tensor(
                    out=s2[:], in0=m2[:, :, 0], in1=m2[:, :, 1], op=OP.add
                )
                mm = tp([P, bn], "mm")
                nc.vector.tensor_tensor(
                    out=mm[:], in0=cf[:, :, 1], in1=cf[:, :, 2], op=OP.mult
                )
                v2 = tp([P, bn], "v2")
                nc.vector.tensor_tensor(out=v2[:], in0=mm[:], in1=selc01, op=OP.mult)
                x = tp([P, bn], "x")
                nc.vector.scalar_tensor_tensor(
                    out=x[:], in0=v2[:], scalar=2.0, in1=s2[:],
                    op0=OP.mult, op1=OP.add,
                )
                bb = tp([P, bn], "bb")
                nc.vector.tensor_tensor(out=bb[:], in0=beta, in1=beta, op=OP.mult)
                pn = tp([P, bn], "pn")
                nc.vector.tensor_tensor(out=pn[:], in0=x[:], in1=bb[:], op=OP.subtract)
                # r = 1/sqrt(pn): bit-magic + 1 Newton step (~0.2% max err,
                # below the bf16 output quantization)
                shv = tinyp.tile([P, bn], U32, name="shv", tag="tiny")
                nc.vector.tensor_scalar(
                    out=shv[:], in0=pn[:].bitcast(U32), scalar1=1, scalar2=None,
                    op0=OP.logical_shift_right,
                )
                r0 = tp([P, bn], "r0")
                nc.vector.tensor_tensor(
                    out=r0[:].bitcast(U32), in0=mg[:].broadcast_to((P, bn)),
                    in1=shv[:], op=OP.subtract,
                )
                a = tp([P, bn], "a")
                nc.vector.tensor_tensor(out=a[:], in0=r0[:], in1=r0[:], op=OP.mult)
                b = tp([P, bn], "b")
                nc.vector.scalar_tensor_tensor(
                    out=b[:], in0=pn[:], scalar=0.5, in1=a[:],
                    op0=OP.mult, op1=OP.mult,
                )
                c = tp([P, bn], "c")
                nc.vector.tensor_scalar(
                    out=c[:], in0=b[:], scalar1=-1.0, scalar2=1.5,
                    op0=OP.mult, op1=OP.add,
                )
                r = tp([P, bn], "r")
                nc.vector.tensor_tensor(out=r[:], in0=c[:], in1=r0[:], op=OP.mult)
                st["cf"], st["r"] = cf, r

            def diags(st, eng):
                # all diag matrices for the batch in ONE broadcast multiply:
                # d[:, 3n+j, :] = eye * cf[n, j]  (j: beta, m0, m1)
                bn, cf = st["bn"], st["cf"]
                d = diagp.tile([P, 3 * bn, P], BF16, name="d", tag="diag")
                cfl = cf[:].rearrange("p c k -> p (c k)")
                eng.tensor_tensor(
                    out=d[:],
                    in0=eyeb[:].rearrange("p (o d) -> p o d", o=1).broadcast_to(
                        (P, 3 * bn, P)
                    ),
                    in1=cfl.broadcast_to((P, 3 * bn, P)),
                    op=OP.mult,
                )
                st["d"] = d
                st["og"] = outp.tile([P, bn, DIM], BF16, name="og", tag="og")

            def combos(st):
                bn, z_g, sel = st["bn"], st["z_g"], st["sel"]
                d, r, og = st["d"], st["r"], st["og"]
                for n in range(bn):
                    pp = psump.tile([P, 2, 512], F32, name="pp", tag="psum")
                    movs = [
                        (0, lambda lo2: z_g[:, n, lo2 : lo2 + 384]),
                        (1, lambda lo2: sel[:, n, lo2 : lo2 + 384]),
                        (2, lambda lo2: sel[:, n, DIM + lo2 : DIM + lo2 + 384]),
                    ]
                    for si, (j, mov) in enumerate(movs):
                        for h2 in range(2):
                            nc.tensor.matmul(
                                pp[:, h2, 0:384], d[:, 3 * n + j, :],
                                mov(h2 * 384),
                                start=(si == 0), stop=(si == 2),
                            )
                    # evacuate PSUM scaled by 1/|p|: mostly ACT (Copy with
                    # per-partition scale), some DVE (GPSIMD cannot touch PSUM)
                    og2 = og[:, n, :].rearrange("p (a d) -> p a d", a=2)
                    if st["lo"] >= 12:
                        nc.vector.tensor_scalar(
                            out=og2, in0=pp[:, :, 0:384],
                            scalar1=r[:, n : n + 1], scalar2=None, op0=OP.mult,
                        )
                    else:
                        nc.scalar.activation(
                            out=og2, in_=pp[:, :, 0:384], func=AT.Copy,
                            scale=r[:, n : n + 1],
                        )

            def flush(st):
                lo, bn = st["lo"], st["bn"]
                nc.sync.dma_start(out=out[:, lo : lo + bn, :], in_=st["og"])

            nc.sync.dma_start(out=idxt[:], in_=idx[:])
            nc.sync.dma_start(out=eyeb[:], in_=eye[:])
            nc.sync.dma_start(out=mg[:], in_=magic[:])
            sts = {}
            for i, (lo, hi) in enumerate(BATCHES):
                sts[i] = loads(lo, hi)
            dots(sts[0])
            smalls(sts[0])
            diags(sts[0], nc.gpsimd)
            dots(sts[1])
            combos(sts[0])
            flush(sts[0])
            smalls(sts[1])
            diags(sts[1], nc.gpsimd)
            dots(sts[2])
            combos(sts[1])
            flush(sts[1])
            smalls(sts[2])
            diags(sts[2], nc.vector)
            combos(sts[2])
            flush(sts[2])
    return nc


_NC_CACHE = None


def _get_nc():
    global _NC_CACHE
    if _NC_CACHE is None:
        _NC_CACHE = build_nc()
    return _NC_CACHE


def build_in_maps(inputs):
    import ml_dtypes

    z = np.asarray(inputs["z"], dtype=np.float32)
    mask = np.asarray(inputs["support_sets_mask"], dtype=np.float32)
    S = np.asarray(inputs["SUPPORT_SETS"], dtype=np.float32)
    A = np.asarray(inputs["ALPHAS"], dtype=np.float32)
    LG = np.asarray(inputs["LOGGAMMA"], dtype=np.float32)

    # partition-major z: zp[core][p, t, :] = z[core*ROWS + t*128 + p]
    zb = (
        z.astype(ml_dtypes.bfloat16)
        .reshape(NCORES, NT, P, DIM)
        .transpose(0, 2, 1, 3)
    )
    idx_full = np.argmax(mask, axis=1).astype(np.uint32)
    idxp = idx_full.reshape(NCORES, NT, P).transpose(0, 2, 1)
    # table rows: [s0 | s1 | C0 C1 2g0 2g1 c01 | pad]
    g = np.exp(LG)  # [K,2]
    C = A * g * np.exp(-2.0 * g)
    c01 = np.sum(S[:, :DIM] * S[:, DIM:], axis=1, keepdims=True)
    tblf = np.zeros((K, TBL_W), dtype=np.float32)
    tblf[:, : 2 * DIM] = S
    tblf[:, PC : PC + 2] = C
    tblf[:, PC + 2 : PC + 4] = 2.0 * g
    tblf[:, PC + 4 : PC + 5] = c01
    tbl = tblf.astype(ml_dtypes.bfloat16)
    eye = np.eye(P, dtype=np.float32).astype(ml_dtypes.bfloat16)
    magic = np.full((P, 1), MAGIC, dtype=np.uint32)

    return [
        {
            "zin": np.ascontiguousarray(zb[c]),
            "idx": np.ascontiguousarray(idxp[c]),
            "tbl": tbl,
            "eye": eye,
            "magic": magic,
        }
        for c in range(NCORES)
    ]


def kernel(support_sets_mask, z, SUPPORT_SETS, ALPHAS, LOGGAMMA):
    in_maps = build_in_maps(
        dict(
            support_sets_mask=support_sets_mask, z=z,
            SUPPORT_SETS=SUPPORT_SETS, ALPHAS=ALPHAS, LOGGAMMA=LOGGAMMA,
        )
    )
    nc = _get_nc()
    res = run_bass_kernel_spmd(nc, in_maps, list(range(NCORES)))
    # out[core] is [P, NT, DIM] partition-major -> rows t*128+p
    return np.concatenate(
        [
            res.results[c]["out"].transpose(1, 0, 2).reshape(ROWS, DIM)
            for c in range(NCORES)
        ],
        axis=0,
    ).astype(np.float32)


# revision 35
# speedup vs baseline: 1.0504x; 1.0504x over previous
"""Trainium2 Bass kernel for CorpusSupportSets RBF tangent-field.

Math per sample row i (dim 768), one-hot mask selects dipole k:
    t_j  = z . s_j                      (unit z, unit s_j)
    m_j  = a_j g_j e^{-g_j(2-2t_j)} = C_j exp(2 g_j t_j),  C_j = a_j g_j e^{-2 g_j}
    beta = -(m0 t0 + m1 t1)
    p    = beta z + m0 s0 + m1 s1
    |p|^2 = m0^2 + m1^2 - beta^2 + 2 m0 m1 (s0.s1)
    out  = p / |p|

Sharding: data-parallel over batch across 8 cores (2048 rows each).

Host prep (layout/dtype only + per-table-row constants): z in bf16
partition-major [128, 16, 768]; per-sample table row index as u32
[128, 16]; table rows [s0|s1|C0 C1 2g0 2g1 c01|pad] bf16; output bf16
partition-major, upcast to f32 on host.

Device structure (16 tiles of 128 rows, 4 groups of 4 tiles):
- per group: one sync HWDGE z load, one batched indirect DMA gathering
  4x128 table rows (gpsimd SWDGE).
- dots t_j: scalar_tensor_tensor with accum_out, split DVE/Pool.
- per-group small math on [128,4,2]: exp on ACT (the only ACT use),
  1/|p| via bit-magic rsqrt + 2 Newton steps on DVE.
- p = diag(bp) z + diag(m0p) s0 + diag(m1p) s1 as accumulating PE
  matmuls with diag stationaries built on DVE (eye * per-partition
  scalar); PSUM->SBUF bf16 copies alternate DVE/Pool.
"""
import sys

for _p in ("/opt/trn_rl_repo",):
    if _p not in sys.path:
        sys.path.insert(0, _p)

import numpy as np

import concourse.bass as bass
import concourse.tile as tile
from concourse import mybir
from concourse.bass_utils import run_bass_kernel_spmd
from concourse.vector_clock import ScopedClock

# ---------------------------------------------------------------------------
# Workaround: this walrus build only accepts ONE semaphore wait per
# instruction; the TileContext exit drain accumulates one wait per live
# semaphore lane.  Split overflow waits onto trailing sync-engine NOPs.
_MAX_WAITS = 1


def _split_waits(nc, inst):
    si = inst.sync_info
    if si is None:
        return
    waits = list(si.on_wait)
    if len(waits) <= _MAX_WAITS:
        return
    inst.sync_info = mybir.SyncInfo(
        on_wait=waits[:_MAX_WAITS], on_update=list(si.on_update)
    )
    for i in range(_MAX_WAITS, len(waits), _MAX_WAITS):
        nop = nc.sync.nop(nofuse=True, hint="drain_wait_overflow")
        nop.ins.sync_info = mybir.SyncInfo(
            on_wait=waits[i : i + _MAX_WAITS], on_update=[]
        )


def _patched_drain_and_barrier(self, tick_clock, wait_clock):
    drain_inst = self.nc.sync.drain()
    wait_clock.add_sem_waits(
        drain_inst.ins, ScopedClock({None: tick_clock.global_clock})
    )
    _split_waits(self.nc, drain_inst.ins)
    self.nc.all_engine_barrier()
    assert self.sems is not None
    popped = self.nc._tile_sem_poison_stack.pop()
    assert popped is self._sem_poison
    self.nc.clear_and_free_semaphores(list(self.sems.allocated().values()))
    self.nc.all_engine_barrier()


_orig_commit = tile.TileContext._commit_instruction


def _patched_commit(self, inst, lazy_reg_writes=True):
    si = getattr(inst, "sync_info", None)
    if (
        si is not None
        and si.on_wait
        and len(si.on_wait) > _MAX_WAITS
        and inst.engine != mybir.EngineType.Unassigned
    ):
        waits = list(si.on_wait)
        inst.sync_info = mybir.SyncInfo(
            on_wait=waits[:_MAX_WAITS], on_update=list(si.on_update)
        )
        for _i, _w in enumerate(waits[_MAX_WAITS:]):
            nop = mybir.InstNoOp(
                name=f"{inst.name}_w{_i}",
                engine=inst.engine,
                sync_info=mybir.SyncInfo(on_wait=[_w], on_update=[]),
                bass_nofuse=True,
            )
            self._add_instruction(nop)
    return _orig_commit(self, inst, lazy_reg_writes)


tile.TileContext._drain_and_barrier = _patched_drain_and_barrier
tile.TileContext._commit_instruction = _patched_commit

# ---------------------------------------------------------------------------
BS, K, DIM = 16384, 1000, 768
NCORES = 8
ROWS = BS // NCORES  # 2048 rows per core
P = 128
NT = ROWS // P  # 16 tiles of 128 rows
GRP = 4  # tiles per group
NG = NT // GRP  # 4 groups
TBL_W = 2 * DIM + 8  # 1544 bf16 cols = 3088B per row
PC = 2 * DIM  # C0, C1, 2g0, 2g1, c01 start here
F32 = mybir.dt.float32
BF16 = mybir.dt.bfloat16
U32 = mybir.dt.uint32
MAGIC = 0x5F3759DF


def build_nc(rows=ROWS):
    NT = rows // P
    HGRP = 8  # tiles per half
    OP = mybir.AluOpType
    AT = mybir.ActivationFunctionType
    nc = bass.Bass()
    zin = nc.dram_tensor("zin", [P, NT, DIM], BF16, kind="ExternalInput")
    idx = nc.dram_tensor("idx", [P, NT], U32, kind="ExternalInput")
    tbl = nc.dram_tensor("tbl", [K, TBL_W], BF16, kind="ExternalInput")
    eye = nc.dram_tensor("eye", [P, P], BF16, kind="ExternalInput")
    magic = nc.dram_tensor("magic", [P, 1], U32, kind="ExternalInput")
    out = nc.dram_tensor("out", [P, NT, DIM], BF16, kind="ExternalOutput")

    with tile.TileContext(nc) as tc:
        with (
            tc.tile_pool(name="zp", bufs=3) as zp,
            tc.tile_pool(name="selp", bufs=3) as selp,
            tc.tile_pool(name="outp", bufs=3) as outp,
            tc.tile_pool(name="scrap", bufs=4) as scrp,
            tc.tile_pool(name="diag", bufs=3) as diagp,
            tc.tile_pool(name="psum", bufs=4, space="PSUM") as psump,
            tc.tile_pool(name="tiny", bufs=40) as tinyp,
            tc.tile_pool(name="singles", bufs=1) as singles,
        ):
            t_a = singles.tile([P, NT, 2], F32)
            idxt = singles.tile([P, NT], U32)
            eyeb = singles.tile([P, P], BF16)
            mg = singles.tile([P, 1], U32)

            # batches of tiles: small first batch so its ACT accumulates /
            # exp / first matmuls start early, small last batch so the
            # serial tail (smalls -> diag -> matmul -> copy -> DMA) is short
            BATCHES = [(0, 6), (6, 12), (12, 16)]

            def loads(lo, hi):
                bn = hi - lo
                z_g = zp.tile([P, bn, DIM], BF16, name="z_g", tag="z")
                nc.sync.dma_start(out=z_g[:], in_=zin[:, lo:hi, :])
                sel = selp.tile([P, bn, TBL_W], BF16, name="sel", tag="sel")
                # multi-column offset APs mis-address on real HW (CoreSim
                # accepts them): one indirect DMA per 128-row tile
                for n in range(bn):
                    t = lo + n
                    nc.gpsimd.indirect_dma_start(
                        out=sel[:, n, :],
                        out_offset=None,
                        in_=tbl[:],
                        in_offset=bass.IndirectOffsetOnAxis(
                            ap=idxt[:, t : t + 1], axis=0
                        ),
                    )
                return dict(lo=lo, bn=bn, z_g=z_g, sel=sel)

            def dots(st):
                lo, bn, z_g, sel = st["lo"], st["bn"], st["z_g"], st["sel"]
                for n in range(bn):
                    t = lo + n
                    # ~6/16 tiles offload both dot accumulates to ACT (2x-mode
                    # DVE multiply + ACT Copy-with-accum); the rest are 1x
                    # stt-with-accum on DVE. Balances DVE vs ACT busy time.
                    # block-contiguous engine assignment so no tile region
                    # is written by two engines: tiles 0-9 ACT accums,
                    # tiles 10-15 DVE stt (ACT is the denser critical path)
                    on_act = t < 10
                    for pole in range(2):
                        if on_act:
                            prod = scrp.tile([P, DIM], BF16, name="prod", tag="scr")
                            nc.vector.tensor_tensor(
                                out=prod[:],
                                in0=z_g[:, n, :],
                                in1=sel[:, n, pole * DIM : (pole + 1) * DIM],
                                op=OP.mult,
                            )
                            pscr = scrp.tile([P, DIM], BF16, name="pscr", tag="scr2")
                            nc.scalar.activation(
                                out=pscr[:],
                                in_=prod[:],
                                func=AT.Copy,
                                accum_out=t_a[:, t, pole : pole + 1],
                            )
                        else:
                            pscr = scrp.tile([P, DIM], BF16, name="pscr", tag="scr2")
                            nc.vector.scalar_tensor_tensor(
                                out=pscr[:],
                                in0=z_g[:, n, :],
                                scalar=1.0,
                                in1=sel[:, n, pole * DIM : (pole + 1) * DIM],
                                op0=OP.mult,
                                op1=OP.mult,
                                accum_out=t_a[:, t, pole : pole + 1],
                            )

            def smalls(st):
                lo, bn, sel = st["lo"], st["bn"], st["sel"]
                tg = t_a[:, lo : lo + bn, :]  # [P,bn,2] f32
                selC = sel[:, :, PC : PC + 2]
                selG2 = sel[:, :, PC + 2 : PC + 4]
                selc01 = sel[:, :, PC + 4 : PC + 5].rearrange("p c o -> p (c o)")
                tp = lambda shape, name: tinyp.tile(shape, F32, name=name, tag="tiny")
                # cf holds the combo coefficients [beta, m0, m1] * 1/|p|
                # per tile, interleaved for the batched diag build
                cf = tinyp.tile([P, bn, 3], F32, name="cf", tag="cf")
                u = tp([P, bn, 2], "u")
                nc.vector.tensor_tensor(out=u[:], in0=selG2, in1=tg, op=OP.mult)
                e = tp([P, bn, 2], "e")
                nc.scalar.activation(out=e[:], in_=u[:], func=AT.Exp)
                m = cf[:, :, 1:3]
                nc.vector.tensor_tensor(out=m, in0=selC, in1=e[:], op=OP.mult)
                hh = tp([P, bn, 2], "hh")
                nc.vector.tensor_tensor(out=hh[:], in0=m, in1=tg, op=OP.mult)
                beta = cf[:, :, 0:1].rearrange("p c o -> p (c o)")
                nc.vector.scalar_tensor_tensor(
                    out=beta, in0=hh[:, :, 0], scalar=-1.0, in1=hh[:, :, 1],
                    op0=OP.mult, op1=OP.subtract,
                )
                m2 = tp([P, bn, 2], "m2")
                nc.vector.tensor_tensor(out=m2[:], in0=m, in1=m, op=OP.mult)
                s2 = tp([P, bn], "s2")
                nc.vector.tensor_tensor(
                    out=s2[:], in0=m2[:, :, 0], in1=m2[:, :, 1], op=OP.add
                )
                mm = tp([P, bn], "mm")
                nc.vector.tensor_tensor(
                    out=mm[:], in0=cf[:, :, 1], in1=cf[:, :, 2], op=OP.mult
                )
                v2 = tp([P, bn], "v2")
                nc.vector.tensor_tensor(out=v2[:], in0=mm[:], in1=selc01, op=OP.mult)
                x = tp([P, bn], "x")
                nc.vector.scalar_tensor_tensor(
                    out=x[:], in0=v2[:], scalar=2.0, in1=s2[:],
                    op0=OP.mult, op1=OP.add,
                )
                bb = tp([P, bn], "bb")
                nc.vector.tensor_tensor(out=bb[:], in0=beta, in1=beta, op=OP.mult)
                pn = tp([P, bn], "pn")
                nc.vector.tensor_tensor(out=pn[:], in0=x[:], in1=bb[:], op=OP.subtract)
                # r = 1/sqrt(pn): bit-magic + 1 Newton step (~0.2% max err,
                # below the bf16 output quantization)
                shv = tinyp.tile([P, bn], U32, name="shv", tag="tiny")
                nc.vector.tensor_scalar(
                    out=shv[:], in0=pn[:].bitcast(U32), scalar1=1, scalar2=None,
                    op0=OP.logical_shift_right,
                )
                r0 = tp([P, bn], "r0")
                nc.vector.tensor_tensor(
                    out=r0[:].bitcast(U32), in0=mg[:].broadcast_to((P, bn)),
                    in1=shv[:], op=OP.subtract,
                )
                a = tp([P, bn], "a")
                nc.vector.tensor_tensor(out=a[:], in0=r0[:], in1=r0[:], op=OP.mult)
                b = tp([P, bn], "b")
                nc.vector.scalar_tensor_tensor(
                    out=b[:], in0=pn[:], scalar=0.5, in1=a[:],
                    op0=OP.mult, op1=OP.mult,
                )
                c = tp([P, bn], "c")
                nc.vector.tensor_scalar(
                    out=c[:], in0=b[:], scalar1=-1.0, scalar2=1.5,
                    op0=OP.mult, op1=OP.add,
                )
                r = tp([P, bn], "r")
                nc.vector.tensor_tensor(out=r[:], in0=c[:], in1=r0[:], op=OP.mult)
                st["cf"], st["r"] = cf, r

            def diags(st, eng):
                # all diag matrices for the batch in ONE broadcast multiply:
                # d[:, 3n+j, :] = eye * cf[n, j]  (j: beta, m0, m1)
                bn, cf = st["bn"], st["cf"]
                d = diagp.tile([P, 3 * bn, P], BF16, name="d", tag="diag")
                cfl = cf[:].rearrange("p c k -> p (c k)")
                eng.tensor_tensor(
                    out=d[:],
                    in0=eyeb[:].rearrange("p (o d) -> p o d", o=1).broadcast_to(
                        (P, 3 * bn, P)
                    ),
                    in1=cfl.broadcast_to((P, 3 * bn, P)),
                    op=OP.mult,
                )
                st["d"] = d
                st["og"] = outp.tile([P, bn, DIM], BF16, name="og", tag="og")

            def combos(st):
                bn, z_g, sel = st["bn"], st["z_g"], st["sel"]
                d, r, og = st["d"], st["r"], st["og"]
                for n in range(bn):
                    pp = psump.tile([P, 2, 512], F32, name="pp", tag="psum")
                    movs = [
                        (0, lambda lo2: z_g[:, n, lo2 : lo2 + 384]),
                        (1, lambda lo2: sel[:, n, lo2 : lo2 + 384]),
                        (2, lambda lo2: sel[:, n, DIM + lo2 : DIM + lo2 + 384]),
                    ]
                    for si, (j, mov) in enumerate(movs):
                        for h2 in range(2):
                            nc.tensor.matmul(
                                pp[:, h2, 0:384], d[:, 3 * n + j, :],
                                mov(h2 * 384),
                                start=(si == 0), stop=(si == 2),
                            )
                    # evacuate PSUM scaled by 1/|p|: mostly ACT (Copy with
                    # per-partition scale), some DVE (GPSIMD cannot touch PSUM)
                    og2 = og[:, n, :].rearrange("p (a d) -> p a d", a=2)
                    if st["lo"] >= 12:
                        nc.vector.tensor_scalar(
                            out=og2, in0=pp[:, :, 0:384],
                            scalar1=r[:, n : n + 1], scalar2=None, op0=OP.mult,
                        )
                    else:
                        nc.scalar.activation(
                            out=og2, in_=pp[:, :, 0:384], func=AT.Copy,
                            scale=r[:, n : n + 1],
                        )

            def flush(st):
                lo, bn = st["lo"], st["bn"]
                nc.sync.dma_start(out=out[:, lo : lo + bn, :], in_=st["og"])

            nc.sync.dma_start(out=idxt[:], in_=idx[:])
            nc.sync.dma_start(out=eyeb[:], in_=eye[:])
            nc.sync.dma_start(out=mg[:], in_=magic[:])
            sts = {}
            for i, (lo, hi) in enumerate(BATCHES):
                sts[i] = loads(lo, hi)
            # diags on DVE: the Pool queue is serialized behind all 16
            # gather descriptor-generations, which stalled the first
            # matmul to t~39us when diag(b0) lived there
            dots(sts[0])
            smalls(sts[0])
            diags(sts[0], nc.vector)
            dots(sts[1])
            combos(sts[0])
            flush(sts[0])
            smalls(sts[1])
            diags(sts[1], nc.vector)
            dots(sts[2])
            combos(sts[1])
            flush(sts[1])
            smalls(sts[2])
            diags(sts[2], nc.vector)
            combos(sts[2])
            flush(sts[2])
    return nc


_NC_CACHE = None


def _get_nc():
    global _NC_CACHE
    if _NC_CACHE is None:
        _NC_CACHE = build_nc()
    return _NC_CACHE


def build_in_maps(inputs):
    import ml_dtypes

    z = np.asarray(inputs["z"], dtype=np.float32)
    mask = np.asarray(inputs["support_sets_mask"], dtype=np.float32)
    S = np.asarray(inputs["SUPPORT_SETS"], dtype=np.float32)
    A = np.asarray(inputs["ALPHAS"], dtype=np.float32)
    LG = np.asarray(inputs["LOGGAMMA"], dtype=np.float32)

    # partition-major z: zp[core][p, t, :] = z[core*ROWS + t*128 + p]
    zb = (
        z.astype(ml_dtypes.bfloat16)
        .reshape(NCORES, NT, P, DIM)
        .transpose(0, 2, 1, 3)
    )
    idx_full = np.argmax(mask, axis=1).astype(np.uint32)
    idxp = idx_full.reshape(NCORES, NT, P).transpose(0, 2, 1)
    # table rows: [s0 | s1 | C0 C1 2g0 2g1 c01 | pad]
    g = np.exp(LG)  # [K,2]
    C = A * g * np.exp(-2.0 * g)
    c01 = np.sum(S[:, :DIM] * S[:, DIM:], axis=1, keepdims=True)
    tblf = np.zeros((K, TBL_W), dtype=np.float32)
    tblf[:, : 2 * DIM] = S
    tblf[:, PC : PC + 2] = C
    tblf[:, PC + 2 : PC + 4] = 2.0 * g
    tblf[:, PC + 4 : PC + 5] = c01
    tbl = tblf.astype(ml_dtypes.bfloat16)
    eye = np.eye(P, dtype=np.float32).astype(ml_dtypes.bfloat16)
    magic = np.full((P, 1), MAGIC, dtype=np.uint32)

    return [
        {
            "zin": np.ascontiguousarray(zb[c]),
            "idx": np.ascontiguousarray(idxp[c]),
            "tbl": tbl,
            "eye": eye,
            "magic": magic,
        }
        for c in range(NCORES)
    ]


def kernel(support_sets_mask, z, SUPPORT_SETS, ALPHAS, LOGGAMMA):
    in_maps = build_in_maps(
        dict(
            support_sets_mask=support_sets_mask, z=z,
            SUPPORT_SETS=SUPPORT_SETS, ALPHAS=ALPHAS, LOGGAMMA=LOGGAMMA,
        )
    )
    nc = _get_nc()
    res = run_bass_kernel_spmd(nc, in_maps, list(range(NCORES)))
    # out[core] is [P, NT, DIM] partition-major -> rows t*128+p
    return np.concatenate(
        [
            res.results[c]["out"].transpose(1, 0, 2).reshape(ROWS, DIM)
            for c in range(NCORES)
        ],
        axis=0,
    ).astype(np.float32)
